# revision 14
# baseline (speedup 1.0000x reference)
"""GCNEncoder (GCNConv + TransformerEncoderLayer) on 8 Trainium2 NeuronCores.

Sharding: nodes are split 512/core (8 cores). Per core:
  - GCN: dense normalized-adjacency block A^T [4096 src, 512 dst] built on
    device via GPSIMD local_scatter from host-permuted (index-only) edge
    layouts; aggregation is a dense fp16 matmul against the AllGathered
    scaled features.
  - Attention: both heads, q = the core's 512 nodes vs all 4096 keys.
    Scores computed transposed (S^T[k,q]) so softmax denominators come from
    a ones-matmul and PV needs no transposes; softmax skips max-subtraction
    (scores are O(1) for this model family; exp cannot overflow fp32).
  - FFN + both LayerNorms fully local.
Two AllGathers (scaled GCN features, hidden-state transpose) are the only
collectives. All matmul operands fp16, accumulation fp32 in PSUM.
"""

import math

import numpy as np

import concourse.bacc as bacc
import concourse.mybir as mybir
import concourse.tile as tile
from concourse import library_config
from concourse.tile_rust import add_dep_helper

N_CORES = 8
N = 4096
E = 131072
DIN = 512
D = 256
H = 2
DH = 128
DFF = 2048
EPS = 1e-5
P = 128

NPC = N // N_CORES          # nodes per core = 512
MPC = NPC // P              # m-chunks per core = 4
KT = N // P                 # src k-tiles = 32
KPAD = 32                   # max out-edges per (core, src-node)
KBD = 80                    # max in-edges per dst node
NDUP = 256                  # max duplicate-edge occurrences per core
DT16 = mybir.dt.float16
DT32 = mybir.dt.float32
DTI16 = mybir.dt.int16
F = mybir.ActivationFunctionType
A = mybir.AluOpType
INV_SQRT_DH = 1.0 / math.sqrt(DH)


def build_kernel():
    nc = bacc.Bacc("TRN2", target_bir_lowering=False, debug=False,
                   num_devices=N_CORES)

    def din(name, shape, dt=DT32):
        return nc.dram_tensor(name, shape, dt, kind="ExternalInput")

    xT_d = din("xT", [P, MPC * DIN], DT16)
    wg_d = din("wg", [P, (DIN // P) * D], DT16)
    warr_d = din("warr", [P, KT * KPAD], DT16)
    idx_d = din("idx", [P, KT * KPAD], DTI16)
    wbd_d = din("wbd", [P, MPC * KBD], DT16)
    dupsr_d = din("dupsr", [P, NDUP // P])
    dupfc_d = din("dupfc", [P, NDUP // P])
    dupw_d = din("dupw", [P, NDUP // P])
    iota1024_d = din("iota1024", [P, KT * KPAD], DT16)
    iota128_d = din("iota128", [P, P], DT16)
    ident_d = din("ident", [P, P])
    winT_d = din("winT", [P, 2 * 3 * D], DT16)
    ipb_d = din("ipb", [P, 6])
    woT_d = din("woT", [P, 2 * D], DT16)
    w1T_d = din("w1T", [P, 2 * DFF], DT16)
    b1_d = din("b1", [P, DFF // P])
    w2T_d = din("w2T", [P, (DFF // P) * D], DT16)
    bias_d = din("bias", [1, 7 * D])

    out_d = nc.dram_tensor("out", [NPC, D], DT32, kind="ExternalOutput")

    with tile.TileContext(nc) as tc:
        with (
            tc.tile_pool(name="keep", bufs=1) as keep,
            tc.tile_pool(name="dram", bufs=1, space="DRAM") as dram,
        ):
            def load16(dram_t, cols):
                f16 = keep.tile([P, cols], DT16, tag=f"ld_{dram_t.name}",
                                name=f"{dram_t.name}16")
                nc.sync.dma_start(f16[:], dram_t[:])
                return f16

            def bc4(ap_2d):
                """[128, D] bias slice -> broadcast [128, MPC, D]."""
                return ap_2d[:, None, :].to_broadcast([P, MPC, D])

            ones16_col = keep.tile([P, 1], DT16)
            ones16_row = keep.tile([1, P], DT16)
            ones32_row = keep.tile([1, P], DT32)
            nc.vector.memset(ones16_col[:], 1.0)
            nc.vector.memset(ones16_row[:], 1.0)
            nc.vector.memset(ones32_row[:], 1.0)

            lib = nc.gpsimd.load_library(library_config.local_scatter)

            gk = ctx_gcn = tc.tile_pool(name="gcn_keep", bufs=1)
            gk = ctx_gcn.__enter__()

            # ---- degrees -> dinv ----
            wbd = gk.tile([P, MPC * KBD], DT16)
            nc.sync.dma_start(wbd[:], wbd_d[:])
            dinv = gk.tile([P, MPC], DT32)
            dinv2 = gk.tile([P, MPC], DT32)
            deg = gk.tile([P, MPC], DT32)
            nc.vector.tensor_reduce(
                deg[:], wbd[:].rearrange("p (m k) -> p m k", k=KBD),
                axis=mybir.AxisListType.X, op=A.add)
            sqd = gk.tile([P, MPC], DT32)
            nc.scalar.activation(sqd[:], deg[:], F.Sqrt, bias=1.0, scale=1.0)
            nc.vector.reciprocal(dinv[:], sqd[:])
            nc.vector.tensor_mul(dinv2[:], dinv[:], dinv[:])

            # ---- xw = x @ W_gcn ----
            xT16 = load16(xT_d, MPC * DIN)
            wg16 = load16(wg_d, (DIN // P) * D)
            xws16 = gk.tile([P, MPC * D], DT16)
            self32 = gk.tile([P, MPC * D], DT32)
            with tc.tile_pool(name="xw_ps", bufs=2, space="PSUM") as xps:
                for m in range(MPC):
                    pxw = xps.tile([P, D], DT32, space="PSUM", tag="xw")
                    for k in range(DIN // P):
                        nc.tensor.matmul(
                            pxw[:],
                            lhsT=xT16[:, DIN * k + P * m:DIN * k + P * m + P],
                            rhs=wg16[:, D * k:D * (k + 1)],
                            start=(k == 0), stop=(k == DIN // P - 1))
                    nc.vector.tensor_scalar(xws16[:, D * m:D * (m + 1)], pxw[:],
                                            dinv[:, m:m + 1], None, op0=A.mult)
                    nc.vector.tensor_scalar(self32[:, D * m:D * (m + 1)], pxw[:],
                                            dinv2[:, m:m + 1], None, op0=A.mult)

            # AllGather xw_s — triggered from ACT so it bypasses the busy
            # GpSimd queue (collectives keep a consistent cross-rank order
            # via data deps: AG2 depends on AG1's output).
            xws_bounce = dram.tile([NPC, D], DT16)
            xws_gath = dram.tile([N, D], DT16, addr_space="Shared")
            nc.sync.dma_start(
                xws_bounce[:].rearrange("(m p) d -> p m d", p=P),
                xws16[:].rearrange("p (m d) -> p m d", m=MPC))
            ag1 = nc.gpsimd.collective_compute(
                "AllGather", A.bypass,
                replica_groups=[list(range(N_CORES))],
                ins=[xws_bounce.opt()], outs=[xws_gath.opt()])

            # ---- A build first: scatters on GpSimd start ASAP ----
            iota1024 = gk.tile([P, KT * KPAD], DT16)
            iota128 = gk.tile([P, P], DT16)
            warr = gk.tile([P, KT * KPAD], DT16)
            idx_t = gk.tile([P, KT * KPAD], DTI16)
            dupsr = gk.tile([P, NDUP // P], DT32)
            dupfc = gk.tile([P, NDUP // P], DT32)
            dupw = gk.tile([P, NDUP // P], DT32)
            nc.sync.dma_start(warr[:], warr_d[:])
            nc.sync.dma_start(idx_t[:], idx_d[:])
            nc.sync.dma_start(iota1024[:], iota1024_d[:])
            nc.sync.dma_start(iota128[:], iota128_d[:])
            nc.sync.dma_start(dupsr[:], dupsr_d[:])
            nc.sync.dma_start(dupfc[:], dupfc_d[:])
            nc.sync.dma_start(dupw[:], dupw_d[:])

            warr16 = gk.tile([P, KT * KPAD], DT16)
            a_tiles = [gk.tile([P, NPC], DT16, tag=f"A{kt}", name=f"A{kt}")
                       for kt in range(KT)]

            with tc.tile_pool(name="gcn_sb", bufs=2) as gsb, \
                 tc.tile_pool(name="gcn_ps", bufs=2, space="PSUM") as gps:
                mrg_ps = [gps.tile([P, 512], DT32, space="PSUM",
                                   tag=f"mrg{h}", name=f"mrg{h}")
                          for h in range(2)]
                for b in range(NDUP // P):
                    sd = gsb.tile([P, P], DT16, tag="sd")
                    vd = gsb.tile([P, KT * KPAD], DT16, tag="vd")
                    nc.vector.tensor_scalar(sd[:], iota128[:],
                                            dupsr[:, b:b + 1], None,
                                            op0=A.is_equal)
                    nc.vector.tensor_scalar(vd[:], iota1024[:],
                                            dupfc[:, b:b + 1],
                                            dupw[:, b:b + 1],
                                            op0=A.is_equal, op1=A.mult)
                    for h in range(2):
                        nc.tensor.matmul(mrg_ps[h][:], lhsT=sd[:],
                                         rhs=vd[:, 512 * h:512 * h + 512],
                                         start=(b == 0),
                                         stop=(b == NDUP // P - 1))
                for h in range(2):
                    nc.vector.tensor_tensor(warr16[:, 512 * h:512 * h + 512],
                                            warr[:, 512 * h:512 * h + 512],
                                            mrg_ps[h][:], op=A.add)
                for kt in range(KT):
                    ls = nc.gpsimd.local_scatter(
                        a_tiles[kt][:],
                        warr16[:, KPAD * kt:KPAD * (kt + 1)],
                        idx_t[:, KPAD * kt:KPAD * (kt + 1)],
                        channels=P, num_elems=NPC, num_idxs=KPAD,
                    )
                    add_dep_helper(lib.ins, ls.ins, reason="lib before scatter")
                    add_dep_helper(ag1.ins, ls.ins, sync=False,
                                   reason="AG1 trigger first on gpsimd queue")

            # constants for later phases (DMA after critical ones)
            ident = keep.tile([P, P], DT32)
            ipb = keep.tile([P, 6], DT32)
            b1t = keep.tile([P, DFF // P], DT32)
            nc.sync.dma_start(ident[:], ident_d[:])
            nc.sync.dma_start(ipb[:], ipb_d[:])
            nc.sync.dma_start(b1t[:], b1_d[:])
            winT16 = load16(winT_d, 2 * 3 * D)
            woT16 = load16(woT_d, 2 * D)

            bias_row = keep.tile([1, 7 * D], DT32)
            nc.sync.dma_start(bias_row[:], bias_d[:])
            bias_bc = keep.tile([P, 7 * D], DT32)
            with tc.tile_pool(name="ps_b", bufs=2, space="PSUM") as psb:
                for j in range(4):
                    w = 448 if j < 3 else 7 * D - 3 * 448
                    pb = psb.tile([P, 448], DT32, space="PSUM", tag="bb")
                    nc.tensor.matmul(pb[:, :w], lhsT=ones32_row[:],
                                     rhs=bias_row[:, j * 448:j * 448 + w],
                                     start=True, stop=True)
                    nc.vector.tensor_copy(bias_bc[:, j * 448:j * 448 + w],
                                          pb[:, :w])
            bgcn_bc = bias_bc[:, 0:D]
            bo_bc = bias_bc[:, D:2 * D]
            b2_bc = bias_bc[:, 2 * D:3 * D]
            ln1g_bc = bias_bc[:, 3 * D:4 * D]
            ln1b_bc = bias_bc[:, 4 * D:5 * D]
            ln2g_bc = bias_bc[:, 5 * D:6 * D]
            ln2b_bc = bias_bc[:, 6 * D:7 * D]

            # ---- aggregation ----
            h_t = keep.tile([P, MPC * D], DT32)
            hT16 = keep.tile([P, 2 * NPC], DT16)
            with tc.tile_pool(name="agg_sb", bufs=1) as asb, \
                 tc.tile_pool(name="agg_ps", bufs=1, space="PSUM") as aps:
                agg_ps = [aps.tile([P, D], DT32, space="PSUM",
                                   tag=f"agg{m}", name=f"agg{m}")
                          for m in range(MPC)]
                xwg = asb.tile([P, KT * D], DT16, tag="xwg", name="xwg")
                nc.sync.dma_start(
                    xwg[:].rearrange("p (t d) -> p t d", t=KT),
                    xws_gath[:].rearrange("(t p) d -> p t d", p=P))
                for kt in range(KT):
                    for m in range(MPC):
                        nc.tensor.matmul(
                            agg_ps[m][:],
                            lhsT=a_tiles[kt][:, P * m:P * (m + 1)],
                            rhs=xwg[:, D * kt:D * (kt + 1)],
                            start=(kt == 0), stop=(kt == KT - 1))

                # h = relu(dinv*agg + self + b_gcn)   (batched epilogue)
                x_all = asb.tile([P, MPC * D], DT32, tag="xall")
                for m in range(MPC):
                    nc.vector.scalar_tensor_tensor(
                        x_all[:, D * m:D * (m + 1)], agg_ps[m][:],
                        dinv[:, m:m + 1], self32[:, D * m:D * (m + 1)],
                        op0=A.mult, op1=A.add)
                nc.vector.tensor_tensor(
                    x_all[:].rearrange("p (m d) -> p m d", m=MPC),
                    x_all[:].rearrange("p (m d) -> p m d", m=MPC),
                    bc4(bgcn_bc), op=A.add)
                nc.scalar.activation(h_t[:], x_all[:], F.Relu)

            # transpose h -> hT16 (local feature-major)
            with tc.tile_pool(name="tr_ps", bufs=2, space="PSUM") as tps:
                for m in range(MPC):
                    for f in range(2):
                        ptr = tps.tile([P, P], DT32, space="PSUM", tag="tr")
                        nc.tensor.transpose(
                            ptr[:], h_t[:, D * m + P * f:D * m + P * (f + 1)],
                            ident[:])
                        nc.vector.tensor_copy(
                            hT16[:, NPC * f + P * m:NPC * f + P * (m + 1)],
                            ptr[:])

            ctx_gcn.__exit__(None, None, None)

            # ---- local K^T / V / Q^T, then ONE packed KV AllGather ----
            # kv rows: 0:128 K^T h0 | 128:256 K^T h1 | 256:384 V h0 | 384:512 V h1
            # (V packed as [128, m*128+d] = natural [512, 128] per head)
            qT16 = keep.tile([P, H * NPC], DT16)
            kv_sb = keep.tile([P, 4 * NPC], DT16)
            with tc.tile_pool(name="kv_ps", bufs=3, space="PSUM") as kvps:
                for h in range(H):
                    pq = kvps.tile([P, NPC], DT32, space="PSUM", tag="kv")
                    for k in range(2):
                        nc.tensor.matmul(
                            pq[:],
                            lhsT=winT16[:, 768 * k + P * h:768 * k + P * (h + 1)],
                            rhs=hT16[:, NPC * k:NPC * (k + 1)],
                            start=(k == 0), stop=(k == 1))
                    nc.vector.tensor_scalar(
                        qT16[:, NPC * h:NPC * (h + 1)], pq[:],
                        ipb[:, h:h + 1], None, op0=A.add)
                    pk = kvps.tile([P, NPC], DT32, space="PSUM", tag="kv")
                    for k in range(2):
                        nc.tensor.matmul(
                            pk[:],
                            lhsT=winT16[:, 768 * k + D + P * h:
                                        768 * k + D + P * (h + 1)],
                            rhs=hT16[:, NPC * k:NPC * (k + 1)],
                            start=(k == 0), stop=(k == 1))
                    nc.vector.tensor_scalar(
                        kv_sb[:, NPC * h:NPC * (h + 1)], pk[:],
                        ipb[:, 2 + h:3 + h], None, op0=A.add)
                    for m in range(MPC):
                        pv = kvps.tile([P, P], DT32, space="PSUM", tag="kvv")
                        for k in range(2):
                            nc.tensor.matmul(
                                pv[:],
                                lhsT=hT16[:, NPC * k + P * m:NPC * k + P * (m + 1)],
                                rhs=winT16[:, 768 * k + 2 * D + P * h:
                                            768 * k + 2 * D + P * (h + 1)],
                                start=(k == 0), stop=(k == 1))
                        nc.vector.tensor_copy(
                            kv_sb[:, NPC * (2 + h) + P * m:
                                  NPC * (2 + h) + P * (m + 1)], pv[:])

            kv_bounce = dram.tile([4 * P, NPC], DT16)
            kv_gath = dram.tile([N_CORES * 4 * P, NPC], DT16,
                                addr_space="Shared")
            nc.sync.dma_start(
                kv_bounce[:].rearrange("(x p) n -> p x n", p=P),
                kv_sb[:].rearrange("p (x n) -> p x n", x=4))
            nc.gpsimd.collective_compute(
                "AllGather", A.bypass,
                replica_groups=[list(range(N_CORES))],
                ins=[kv_bounce.opt()], outs=[kv_gath.opt()])

            # FFN weights stream during the AllGather
            w1T16 = load16(w1T_d, 2 * DFF)
            w2T16 = load16(w2T_d, (DFF // P) * D)

            # ---- load gathered K^T / V ----
            kT16 = keep.tile([P, H * N], DT16)
            v16 = keep.tile([P, H * N], DT16)
            gv = kv_gath[:].rearrange("(g x p) n -> x p g n",
                                      g=N_CORES, x=4, p=P)
            for h in range(H):
                nc.sync.dma_start(
                    kT16[:, N * h:N * (h + 1)].rearrange(
                        "p (g n) -> p g n", g=N_CORES), gv[h])
                nc.sync.dma_start(
                    v16[:, N * h:N * (h + 1)].rearrange(
                        "p (g n) -> p g n", g=N_CORES), gv[2 + h])

            # ---- S^T -> exp -> PV + sums ----
            oT16 = keep.tile([P, H * NPC], DT16)
            with tc.tile_pool(name="att_sb", bufs=3) as atsb, \
                 tc.tile_pool(name="att_ps", bufs=1, space="PSUM") as atps, \
                 tc.tile_pool(name="s_ps", bufs=3, space="PSUM") as sps:
                o_ps = [atps.tile([P, NPC], DT32, space="PSUM",
                                  tag=f"o{h}", name=f"o{h}")
                        for h in range(H)]
                sum_ps = [atps.tile([1, NPC], DT32, space="PSUM",
                                    tag=f"sm{h}", name=f"sm{h}")
                          for h in range(H)]
                for kt in range(KT):
                    for h in range(H):
                        ps_s = sps.tile([P, NPC], DT32, space="PSUM", tag="S")
                        nc.tensor.matmul(
                            ps_s[:],
                            lhsT=kT16[:, N * h + P * kt:N * h + P * (kt + 1)],
                            rhs=qT16[:, NPC * h:NPC * (h + 1)],
                            start=True, stop=True)
                        es = atsb.tile([P, NPC], DT16, tag="es")
                        nc.scalar.activation(es[:], ps_s[:], F.Exp,
                                             scale=INV_SQRT_DH)
                        nc.tensor.matmul(
                            o_ps[h][:],
                            lhsT=v16[:, N * h + P * kt:N * h + P * (kt + 1)],
                            rhs=es[:], start=(kt == 0), stop=(kt == KT - 1))
                        nc.tensor.matmul(
                            sum_ps[h][:], lhsT=ones16_col[:], rhs=es[:],
                            start=(kt == 0), stop=(kt == KT - 1))

                for h in range(H):
                    rc = atsb.tile([1, NPC], DT32, tag="rc")
                    nc.vector.reciprocal(rc[:], sum_ps[h][:])
                    rc16 = atsb.tile([1, NPC], DT16, tag="rc16")
                    nc.vector.tensor_copy(rc16[:], rc[:])
                    pr = sps.tile([P, NPC], DT32, space="PSUM", tag="S")
                    nc.tensor.matmul(pr[:], lhsT=ones16_row[:], rhs=rc16[:],
                                     start=True, stop=True)
                    rbc = atsb.tile([P, NPC], DT32, tag="rbc")
                    nc.vector.tensor_copy(rbc[:], pr[:])
                    tmp = atsb.tile([P, NPC], DT32, tag="otmp")
                    nc.vector.tensor_mul(tmp[:], o_ps[h][:], rbc[:])
                    nc.vector.tensor_scalar(
                        oT16[:, NPC * h:NPC * (h + 1)], tmp[:],
                        ipb[:, 4 + h:5 + h], None, op0=A.add)

            # ---- o_proj + residual + LN1 (batched) ----
            h1_t = keep.tile([P, MPC * D], DT32)
            h1T16 = keep.tile([P, 2 * NPC], DT16)
            with tc.tile_pool(name="ln_sb", bufs=2) as lsb, \
                 tc.tile_pool(name="op_ps", bufs=2, space="PSUM") as ops:

                def layernorm_all(dst, x_all, g_sl, b_sl, tag):
                    """LN over feature dim for all MPC chunks at once.
                    x_all/dst: [128, MPC*D] fp32 tiles."""
                    mu4 = lsb.tile([P, MPC], DT32, tag=f"{tag}mu")
                    nc.vector.tensor_reduce(
                        mu4[:], x_all[:].rearrange("p (m d) -> p m d", m=MPC),
                        axis=mybir.AxisListType.X, op=A.add)
                    negmu4 = lsb.tile([P, MPC], DT32, tag=f"{tag}nm")
                    nc.vector.tensor_scalar(negmu4[:], mu4[:], -1.0 / D, None,
                                            op0=A.mult)
                    sq4 = lsb.tile([P, D], DT32, tag=f"{tag}sq")
                    ssq4 = lsb.tile([P, MPC], DT32, tag=f"{tag}ss")
                    for m in range(MPC):
                        nc.scalar.activation(sq4[:], x_all[:, D * m:D * (m + 1)],
                                             F.Square, bias=negmu4[:, m:m + 1],
                                             accum_out=ssq4[:, m:m + 1])
                    var4 = lsb.tile([P, MPC], DT32, tag=f"{tag}vr")
                    nc.vector.tensor_scalar(var4[:], ssq4[:], 1.0 / D, EPS,
                                            op0=A.mult, op1=A.add)
                    sd4 = lsb.tile([P, MPC], DT32, tag=f"{tag}sd")
                    nc.scalar.activation(sd4[:], var4[:], F.Sqrt)
                    rstd4 = lsb.tile([P, MPC], DT32, tag=f"{tag}rs")
                    nc.vector.reciprocal(rstd4[:], sd4[:])
                    xc = lsb.tile([P, MPC * D], DT32, tag=f"{tag}xc")
                    for m in range(MPC):
                        nc.vector.tensor_scalar(
                            xc[:, D * m:D * (m + 1)], x_all[:, D * m:D * (m + 1)],
                            negmu4[:, m:m + 1], rstd4[:, m:m + 1],
                            op0=A.add, op1=A.mult)
                    nc.vector.tensor_tensor(
                        xc[:].rearrange("p (m d) -> p m d", m=MPC),
                        xc[:].rearrange("p (m d) -> p m d", m=MPC),
                        bc4(g_sl), op=A.mult)
                    nc.vector.tensor_tensor(
                        dst[:].rearrange("p (m d) -> p m d", m=MPC),
                        xc[:].rearrange("p (m d) -> p m d", m=MPC),
                        bc4(b_sl), op=A.add)

                x1_all = lsb.tile([P, MPC * D], DT32, tag="x1all")
                for m in range(MPC):
                    pa = ops.tile([P, D], DT32, space="PSUM", tag="op")
                    for h in range(H):
                        nc.tensor.matmul(
                            pa[:],
                            lhsT=oT16[:, NPC * h + P * m:NPC * h + P * (m + 1)],
                            rhs=woT16[:, D * h:D * (h + 1)],
                            start=(h == 0), stop=(h == H - 1))
                    nc.vector.scalar_tensor_tensor(
                        x1_all[:, D * m:D * (m + 1)], pa[:], 1.0,
                        h_t[:, D * m:D * (m + 1)], op0=A.mult, op1=A.add)
                nc.vector.tensor_tensor(
                    x1_all[:].rearrange("p (m d) -> p m d", m=MPC),
                    x1_all[:].rearrange("p (m d) -> p m d", m=MPC),
                    bc4(bo_bc), op=A.add)
                layernorm_all(h1_t, x1_all, ln1g_bc, ln1b_bc, "a")

                with tc.tile_pool(name="tr2_ps", bufs=2, space="PSUM") as tps2:
                    for m in range(MPC):
                        for f in range(2):
                            ptr = tps2.tile([P, P], DT32, space="PSUM",
                                            tag="tr2")
                            nc.tensor.transpose(
                                ptr[:],
                                h1_t[:, D * m + P * f:D * m + P * (f + 1)],
                                ident[:])
                            nc.vector.tensor_copy(
                                h1T16[:, NPC * f + P * m:NPC * f + P * (m + 1)],
                                ptr[:])

                # ---- FFN ----
                out_sb = keep.tile([P, MPC * D], DT32)
                ff1T = keep.tile([P, (DFF // P) * NPC], DT16)
                with tc.tile_pool(name="f1_ps", bufs=3, space="PSUM") as fps:
                    for dc in range(DFF // P):
                        pf = fps.tile([P, NPC], DT32, space="PSUM", tag="f1")
                        for k in range(2):
                            nc.tensor.matmul(
                                pf[:],
                                lhsT=w1T16[:, DFF * k + P * dc:
                                           DFF * k + P * (dc + 1)],
                                rhs=h1T16[:, NPC * k:NPC * (k + 1)],
                                start=(k == 0), stop=(k == 1))
                        nc.scalar.activation(
                            ff1T[:, NPC * dc:NPC * (dc + 1)], pf[:], F.Relu,
                            bias=b1t[:, dc:dc + 1])

                x2_all = lsb.tile([P, MPC * D], DT32, tag="x2all")
                with tc.tile_pool(name="f2_ps", bufs=2, space="PSUM") as fps2:
                    for m in range(MPC):
                        pf2 = fps2.tile([P, D], DT32, space="PSUM", tag="f2")
                        for kt2 in range(DFF // P):
                            nc.tensor.matmul(
                                pf2[:],
                                lhsT=ff1T[:, NPC * kt2 + P * m:
                                          NPC * kt2 + P * (m + 1)],
                                rhs=w2T16[:, D * kt2:D * (kt2 + 1)],
                                start=(kt2 == 0), stop=(kt2 == DFF // P - 1))
                        nc.vector.scalar_tensor_tensor(
                            x2_all[:, D * m:D * (m + 1)], pf2[:], 1.0,
                            h1_t[:, D * m:D * (m + 1)], op0=A.mult, op1=A.add)
                nc.vector.tensor_tensor(
                    x2_all[:].rearrange("p (m d) -> p m d", m=MPC),
                    x2_all[:].rearrange("p (m d) -> p m d", m=MPC),
                    bc4(b2_bc), op=A.add)
                layernorm_all(out_sb, x2_all, ln2g_bc, ln2b_bc, "b")
                nc.sync.dma_start(
                    out_d[:].rearrange("(m p) d -> p m d", p=P),
                    out_sb[:].rearrange("p (m d) -> p m d", m=MPC))

    nc.compile()
    return nc


# ======================= host-side prep =======================

def _prep_inputs(x, edge_index, edge_weight, W_gcn, b_gcn, in_proj_w,
                 in_proj_b, out_proj_w, out_proj_b, lin1_w, lin1_b, lin2_w,
                 lin2_b, ln1_g, ln1_b, ln2_g, ln2_b):
    """Pure index-permutation / layout prep. Returns per-core input maps."""
    x = np.asarray(x, np.float32)
    src = np.asarray(edge_index[0], np.int64)
    dst = np.asarray(edge_index[1], np.int64)
    w = np.asarray(edge_weight, np.float32)

    def wrap128(a):
        # [n*128, m] -> [128, n*m] with col block t <- rows [128t, 128t+128)
        n = a.shape[0] // P
        return np.ascontiguousarray(
            a.reshape(n, P, a.shape[1]).transpose(1, 0, 2).reshape(P, -1))

    iota1024 = np.tile(np.arange(KT * KPAD, dtype=np.float32), (P, 1))
    iota128 = np.tile(np.arange(P, dtype=np.float32), (P, 1))
    ident = np.eye(P, dtype=np.float32)
    bias_stack = np.concatenate([
        np.asarray(v, np.float32).reshape(-1) for v in
        (b_gcn, out_proj_b, lin2_b, ln1_g, ln1_b, ln2_g, ln2_b)
    ]).reshape(1, -1)

    f16 = np.float16
    shared = {
        "wg": wrap128(np.asarray(W_gcn, np.float32)).astype(f16),
        "iota1024": iota1024.astype(f16), "iota128": iota128.astype(f16),
        "ident": ident,
        "winT": wrap128(np.ascontiguousarray(
            np.asarray(in_proj_w, np.float32).T)).astype(f16),
        "ipb": np.ascontiguousarray(
            np.asarray(in_proj_b, np.float32).reshape(6, P).T),
        "woT": wrap128(np.ascontiguousarray(
            np.asarray(out_proj_w, np.float32).T)).astype(f16),
        "w1T": wrap128(np.ascontiguousarray(
            np.asarray(lin1_w, np.float32).T)).astype(f16),
        "b1": np.ascontiguousarray(
            np.asarray(lin1_b, np.float32).reshape(DFF // P, P).T),
        "w2T": wrap128(np.ascontiguousarray(
            np.asarray(lin2_w, np.float32).T)).astype(f16),
        "bias": bias_stack,
    }

    core_of = dst // NPC
    in_maps = []
    for c in range(N_CORES):
        sel = np.nonzero(core_of == c)[0]
        s_c = src[sel]
        d_c = (dst[sel] - NPC * c).astype(np.int64)
        w_c = w[sel]

        w_arr = np.zeros((N, KPAD), np.float32)
        idx_arr = np.full((N, KPAD), -1, np.int16)
        counts = np.zeros(N, np.int32)
        first_slot = {}
        dup_sr, dup_fc, dup_w = [], [], []
        for si, di, wi in zip(s_c.tolist(), d_c.tolist(), w_c.tolist()):
            key = si * NPC + di
            slot = first_slot.get(key)
            if slot is None:
                j = int(counts[si])
                assert j < KPAD, f"KPAD overflow at src {si}"
                counts[si] = j + 1
                w_arr[si, j] = wi
                idx_arr[si, j] = di
                first_slot[key] = j
            else:
                dup_sr.append(si % P)
                dup_fc.append(KPAD * (si // P) + slot)
                dup_w.append(wi)
        assert len(dup_sr) <= NDUP, f"NDUP overflow: {len(dup_sr)}"

        def pad_dup(vals, dtype=np.float32):
            a = np.zeros(NDUP, dtype)
            a[:len(vals)] = vals
            return np.ascontiguousarray(a.reshape(NDUP // P, P).T)

        wbd = np.zeros((NPC, KBD), np.float32)
        cnt2 = np.zeros(NPC, np.int32)
        for di, wi in zip(d_c.tolist(), w_c.tolist()):
            j = int(cnt2[di])
            assert j < KBD, f"KBD overflow at dst {di}"
            wbd[di, j] = wi
            cnt2[di] = j + 1

        in_maps.append({
            **shared,
            "xT": wrap128(np.ascontiguousarray(
                x[NPC * c:NPC * (c + 1)].T)).astype(f16),
            "warr": wrap128(w_arr).astype(f16),
            "idx": wrap128(idx_arr),
            "wbd": wrap128(wbd).astype(f16),
            "dupsr": pad_dup(dup_sr),
            "dupfc": pad_dup(dup_fc),
            "dupw": pad_dup(dup_w),
        })
    return in_maps


# ======================= runner =======================

class _Runner:
    """Persistent-jit SPMD executor (mirrors bass2jax.run_bass_via_pjrt)."""

    def __init__(self, nc):
        import jax
        from jax.sharding import Mesh, PartitionSpec
        from jax.experimental.shard_map import shard_map
        from concourse.bass2jax import (_bass_exec_p, install_neuronx_cc_hook,
                                        partition_id_tensor)
        install_neuronx_cc_hook()
        self.jax = jax
        partition_name = (nc.partition_id_tensor.name
                          if nc.partition_id_tensor else None)
        in_names, out_names, out_avals, zero_outs = [], [], [], []
        for alloc in nc.m.functions[0].allocations:
            if not isinstance(alloc, mybir.MemoryLocationSet):
                continue
            name = alloc.memorylocations[0].name
            if alloc.kind == "ExternalInput":
                if name != partition_name:
                    in_names.append(name)
            elif alloc.kind == "ExternalOutput":
                out_names.append(name)
                shape = tuple(alloc.tensor_shape)
                dtype = mybir.dt.np(alloc.dtype)
                out_avals.append(jax.core.ShapedArray(shape, dtype))
                zero_outs.append(np.zeros(shape, dtype))
        self.in_names, self.out_names = in_names, out_names
        self.out_shapes = [tuple(a.shape) for a in out_avals]
        self.n_params = len(in_names)
        self.zero_outs = zero_outs
        all_in = in_names + out_names
        if partition_name is not None:
            all_in.append(partition_name)

        def _body(*args):
            operands = list(args)
            if partition_name is not None:
                operands.append(partition_id_tensor())
            return tuple(_bass_exec_p.bind(
                *operands, out_avals=tuple(out_avals), in_names=tuple(all_in),
                out_names=tuple(out_names), lowering_input_output_aliases=(),
                sim_require_finite=True, sim_require_nnan=True, nc=nc))

        devices = jax.devices()[:N_CORES]
        self.mesh = Mesh(np.asarray(devices), ("core",))
        nin = self.n_params + len(out_names)
        self.fn = jax.jit(
            shard_map(_body, mesh=self.mesh,
                      in_specs=(PartitionSpec("core"),) * nin,
                      out_specs=(PartitionSpec("core"),) * len(out_names),
                      check_rep=False),
            keep_unused=True)

    def place(self, in_maps):
        import jax
        from jax.sharding import PartitionSpec
        per_core = [[np.asarray(m[n]) for n in self.in_names] for m in in_maps]
        concat = [np.concatenate([per_core[c][i] for c in range(N_CORES)], axis=0)
                  for i in range(self.n_params)]
        zeros = [np.zeros((N_CORES * z.shape[0], *z.shape[1:]), z.dtype)
                 for z in self.zero_outs]
        sh = jax.sharding.NamedSharding(self.mesh, PartitionSpec("core"))
        return [jax.device_put(a, sh) for a in (*concat, *zeros)]

    def run(self, args):
        outs = self.fn(*args)
        self.jax.block_until_ready(outs)
        return outs

    def results(self, outs):
        res = []
        for c in range(N_CORES):
            d = {}
            for i, name in enumerate(self.out_names):
                full = np.asarray(outs[i])
                ps = self.out_shapes[i]
                d[name] = full.reshape((N_CORES,) + ps)[c]
            res.append(d)
        return res


_CACHE = {}


def _get_runner():
    if "runner" not in _CACHE:
        nc = build_kernel()
        _CACHE["nc"] = nc
        _CACHE["runner"] = _Runner(nc)
    return _CACHE["runner"]


def kernel(**inputs) -> np.ndarray:
    runner = _get_runner()
    in_maps = _prep_inputs(**inputs)
    args = runner.place(in_maps)
    outs = runner.run(args)
    res = runner.results(outs)
    return np.concatenate([res[c]["out"] for c in range(N_CORES)], axis=0)


# revision 15
# speedup vs baseline: 1.0618x; 1.0618x over previous
"""GCNEncoder (GCNConv + TransformerEncoderLayer) on 8 Trainium2 NeuronCores.

Sharding: nodes are split 512/core (8 cores). Per core:
  - GCN: dense normalized-adjacency block A^T [4096 src, 512 dst] built on
    device via GPSIMD local_scatter from host-permuted (index-only) edge
    layouts; aggregation is a dense fp16 matmul against the AllGathered
    scaled features.
  - Attention: both heads, q = the core's 512 nodes vs all 4096 keys.
    Scores computed transposed (S^T[k,q]) so softmax denominators come from
    a ones-matmul and PV needs no transposes; softmax skips max-subtraction
    (scores are O(1) for this model family; exp cannot overflow fp32).
  - FFN + both LayerNorms fully local.
Two AllGathers (scaled GCN features, hidden-state transpose) are the only
collectives. All matmul operands fp16, accumulation fp32 in PSUM.
"""

import math

import numpy as np

import concourse.bacc as bacc
import concourse.mybir as mybir
import concourse.tile as tile
from concourse import library_config
from concourse.tile_rust import add_dep_helper

N_CORES = 8
N = 4096
E = 131072
DIN = 512
D = 256
H = 2
DH = 128
DFF = 2048
EPS = 1e-5
P = 128

NPC = N // N_CORES          # nodes per core = 512
MPC = NPC // P              # m-chunks per core = 4
KT = N // P                 # src k-tiles = 32
KPAD = 32                   # max out-edges per (core, src-node)
KBD = 80                    # max in-edges per dst node
NDUP = 256                  # max duplicate-edge occurrences per core
DT16 = mybir.dt.float16
DT32 = mybir.dt.float32
DTI16 = mybir.dt.int16
F = mybir.ActivationFunctionType
A = mybir.AluOpType
INV_SQRT_DH = 1.0 / math.sqrt(DH)


def build_kernel():
    nc = bacc.Bacc("TRN2", target_bir_lowering=False, debug=False,
                   num_devices=N_CORES)

    def din(name, shape, dt=DT32):
        return nc.dram_tensor(name, shape, dt, kind="ExternalInput")

    xT_d = din("xT", [P, MPC * DIN], DT16)
    wg_d = din("wg", [P, (DIN // P) * D], DT16)
    warr_d = din("warr", [P, KT * KPAD], DT16)
    idx_d = din("idx", [P, KT * KPAD], DTI16)
    wbd_d = din("wbd", [P, MPC * KBD], DT16)
    dupsr_d = din("dupsr", [P, NDUP // P])
    dupfc_d = din("dupfc", [P, NDUP // P])
    dupw_d = din("dupw", [P, NDUP // P])
    iota1024_d = din("iota1024", [P, KT * KPAD], DT16)
    iota128_d = din("iota128", [P, P], DT16)
    ident_d = din("ident", [P, P])
    winT_d = din("winT", [P, 2 * 3 * D], DT16)
    ipb_d = din("ipb", [P, 6])
    woT_d = din("woT", [P, 2 * D], DT16)
    w1T_d = din("w1T", [P, 2 * DFF], DT16)
    b1_d = din("b1", [P, DFF // P])
    w2T_d = din("w2T", [P, (DFF // P) * D], DT16)
    bias_d = din("bias", [1, 7 * D])

    out_d = nc.dram_tensor("out", [NPC, D], DT32, kind="ExternalOutput")

    with tile.TileContext(nc) as tc:
        with (
            tc.tile_pool(name="keep", bufs=1) as keep,
            tc.tile_pool(name="dram", bufs=1, space="DRAM") as dram,
        ):
            def load16(dram_t, cols):
                f16 = keep.tile([P, cols], DT16, tag=f"ld_{dram_t.name}",
                                name=f"{dram_t.name}16")
                nc.sync.dma_start(f16[:], dram_t[:])
                return f16

            def bc4(ap_2d):
                """[128, D] bias slice -> broadcast [128, MPC, D]."""
                return ap_2d[:, None, :].to_broadcast([P, MPC, D])

            ones16_col = keep.tile([P, 1], DT16)
            ones16_row = keep.tile([1, P], DT16)
            ones32_row = keep.tile([1, P], DT32)
            nc.vector.memset(ones16_col[:], 1.0)
            nc.vector.memset(ones16_row[:], 1.0)
            nc.vector.memset(ones32_row[:], 1.0)

            lib = nc.gpsimd.load_library(library_config.local_scatter)

            gk = ctx_gcn = tc.tile_pool(name="gcn_keep", bufs=1)
            gk = ctx_gcn.__enter__()

            # ---- A build first: scatters on GpSimd start ASAP ----
            iota1024 = gk.tile([P, KT * KPAD], DT16)
            iota128 = gk.tile([P, P], DT16)
            warr = gk.tile([P, KT * KPAD], DT16)
            idx_t = gk.tile([P, KT * KPAD], DTI16)
            dupsr = gk.tile([P, NDUP // P], DT32)
            dupfc = gk.tile([P, NDUP // P], DT32)
            dupw = gk.tile([P, NDUP // P], DT32)
            nc.sync.dma_start(warr[:], warr_d[:])
            nc.sync.dma_start(idx_t[:], idx_d[:])
            nc.sync.dma_start(iota1024[:], iota1024_d[:])
            nc.sync.dma_start(iota128[:], iota128_d[:])
            nc.sync.dma_start(dupsr[:], dupsr_d[:])
            nc.sync.dma_start(dupfc[:], dupfc_d[:])
            nc.sync.dma_start(dupw[:], dupw_d[:])

            warr16 = gk.tile([P, KT * KPAD], DT16)
            a_tiles = [gk.tile([P, NPC], DT16, tag=f"A{kt}", name=f"A{kt}")
                       for kt in range(KT)]

            with tc.tile_pool(name="gcn_sb", bufs=2) as gsb, \
                 tc.tile_pool(name="gcn_ps", bufs=2, space="PSUM") as gps:
                mrg_ps = [gps.tile([P, 512], DT32, space="PSUM",
                                   tag=f"mrg{h}", name=f"mrg{h}")
                          for h in range(2)]
                for b in range(NDUP // P):
                    sd = gsb.tile([P, P], DT16, tag="sd")
                    vd = gsb.tile([P, KT * KPAD], DT16, tag="vd")
                    nc.vector.tensor_scalar(sd[:], iota128[:],
                                            dupsr[:, b:b + 1], None,
                                            op0=A.is_equal)
                    nc.vector.tensor_scalar(vd[:], iota1024[:],
                                            dupfc[:, b:b + 1],
                                            dupw[:, b:b + 1],
                                            op0=A.is_equal, op1=A.mult)
                    for h in range(2):
                        nc.tensor.matmul(mrg_ps[h][:], lhsT=sd[:],
                                         rhs=vd[:, 512 * h:512 * h + 512],
                                         start=(b == 0),
                                         stop=(b == NDUP // P - 1))
                for h in range(2):
                    nc.vector.tensor_tensor(warr16[:, 512 * h:512 * h + 512],
                                            warr[:, 512 * h:512 * h + 512],
                                            mrg_ps[h][:], op=A.add)
                for kt in range(KT):
                    ls = nc.gpsimd.local_scatter(
                        a_tiles[kt][:],
                        warr16[:, KPAD * kt:KPAD * (kt + 1)],
                        idx_t[:, KPAD * kt:KPAD * (kt + 1)],
                        channels=P, num_elems=NPC, num_idxs=KPAD,
                    )
                    add_dep_helper(lib.ins, ls.ins, reason="lib before scatter")

            # ---- degrees -> dinv ----
            wbd = gk.tile([P, MPC * KBD], DT16)
            nc.sync.dma_start(wbd[:], wbd_d[:])
            dinv = gk.tile([P, MPC], DT32)
            dinv2 = gk.tile([P, MPC], DT32)
            deg = gk.tile([P, MPC], DT32)
            nc.vector.tensor_reduce(
                deg[:], wbd[:].rearrange("p (m k) -> p m k", k=KBD),
                axis=mybir.AxisListType.X, op=A.add)
            sqd = gk.tile([P, MPC], DT32)
            nc.scalar.activation(sqd[:], deg[:], F.Sqrt, bias=1.0, scale=1.0)
            nc.vector.reciprocal(dinv[:], sqd[:])
            nc.vector.tensor_mul(dinv2[:], dinv[:], dinv[:])

            # ---- xw = x @ W_gcn ----
            xT16 = load16(xT_d, MPC * DIN)
            wg16 = load16(wg_d, (DIN // P) * D)
            xws16 = gk.tile([P, MPC * D], DT16)
            self32 = gk.tile([P, MPC * D], DT32)
            with tc.tile_pool(name="xw_ps", bufs=2, space="PSUM") as xps:
                for m in range(MPC):
                    pxw = xps.tile([P, D], DT32, space="PSUM", tag="xw")
                    for k in range(DIN // P):
                        nc.tensor.matmul(
                            pxw[:],
                            lhsT=xT16[:, DIN * k + P * m:DIN * k + P * m + P],
                            rhs=wg16[:, D * k:D * (k + 1)],
                            start=(k == 0), stop=(k == DIN // P - 1))
                    nc.vector.tensor_scalar(xws16[:, D * m:D * (m + 1)], pxw[:],
                                            dinv[:, m:m + 1], None, op0=A.mult)
                    nc.vector.tensor_scalar(self32[:, D * m:D * (m + 1)], pxw[:],
                                            dinv2[:, m:m + 1], None, op0=A.mult)

            # AllGather xw_s — triggered from ACT so it bypasses the busy
            # GpSimd queue (collectives keep a consistent cross-rank order
            # via data deps: AG2 depends on AG1's output).
            xws_bounce = dram.tile([NPC, D], DT16)
            xws_gath = dram.tile([N, D], DT16, addr_space="Shared")
            nc.scalar.dma_start(
                xws_bounce[:].rearrange("(m p) d -> p m d", p=P),
                xws16[:].rearrange("p (m d) -> p m d", m=MPC))
            nc.gpsimd.collective_compute(
                "AllGather", A.bypass,
                replica_groups=[list(range(N_CORES))],
                ins=[xws_bounce.opt()], outs=[xws_gath.opt()])

            # constants for later phases (DMA after critical ones)
            ident = keep.tile([P, P], DT32)
            ipb = keep.tile([P, 6], DT32)
            b1t = keep.tile([P, DFF // P], DT32)
            nc.sync.dma_start(ident[:], ident_d[:])
            nc.sync.dma_start(ipb[:], ipb_d[:])
            nc.sync.dma_start(b1t[:], b1_d[:])
            winT16 = load16(winT_d, 2 * 3 * D)
            woT16 = load16(woT_d, 2 * D)

            bias_row = keep.tile([1, 7 * D], DT32)
            nc.sync.dma_start(bias_row[:], bias_d[:])
            bias_bc = keep.tile([P, 7 * D], DT32)
            with tc.tile_pool(name="ps_b", bufs=2, space="PSUM") as psb:
                for j in range(4):
                    w = 448 if j < 3 else 7 * D - 3 * 448
                    pb = psb.tile([P, 448], DT32, space="PSUM", tag="bb")
                    nc.tensor.matmul(pb[:, :w], lhsT=ones32_row[:],
                                     rhs=bias_row[:, j * 448:j * 448 + w],
                                     start=True, stop=True)
                    nc.vector.tensor_copy(bias_bc[:, j * 448:j * 448 + w],
                                          pb[:, :w])
            bgcn_bc = bias_bc[:, 0:D]
            bo_bc = bias_bc[:, D:2 * D]
            b2_bc = bias_bc[:, 2 * D:3 * D]
            ln1g_bc = bias_bc[:, 3 * D:4 * D]
            ln1b_bc = bias_bc[:, 4 * D:5 * D]
            ln2g_bc = bias_bc[:, 5 * D:6 * D]
            ln2b_bc = bias_bc[:, 6 * D:7 * D]

            # ---- aggregation ----
            h_t = keep.tile([P, MPC * D], DT32)
            hT16 = keep.tile([P, 2 * NPC], DT16)
            with tc.tile_pool(name="agg_sb", bufs=1) as asb, \
                 tc.tile_pool(name="agg_ps", bufs=1, space="PSUM") as aps:
                agg_ps = [aps.tile([P, D], DT32, space="PSUM",
                                   tag=f"agg{m}", name=f"agg{m}")
                          for m in range(MPC)]
                xwg = asb.tile([P, KT * D], DT16, tag="xwg", name="xwg")
                nc.scalar.dma_start(
                    xwg[:].rearrange("p (t d) -> p t d", t=KT),
                    xws_gath[:].rearrange("(t p) d -> p t d", p=P))
                for kt in range(KT):
                    for m in range(MPC):
                        nc.tensor.matmul(
                            agg_ps[m][:],
                            lhsT=a_tiles[kt][:, P * m:P * (m + 1)],
                            rhs=xwg[:, D * kt:D * (kt + 1)],
                            start=(kt == 0), stop=(kt == KT - 1))

                # h = relu(dinv*agg + self + b_gcn)   (batched epilogue)
                x_all = asb.tile([P, MPC * D], DT32, tag="xall")
                for m in range(MPC):
                    nc.vector.scalar_tensor_tensor(
                        x_all[:, D * m:D * (m + 1)], agg_ps[m][:],
                        dinv[:, m:m + 1], self32[:, D * m:D * (m + 1)],
                        op0=A.mult, op1=A.add)
                nc.vector.tensor_tensor(
                    x_all[:].rearrange("p (m d) -> p m d", m=MPC),
                    x_all[:].rearrange("p (m d) -> p m d", m=MPC),
                    bc4(bgcn_bc), op=A.add)
                nc.scalar.activation(h_t[:], x_all[:], F.Relu)

            # transpose h -> hT16 (local feature-major)
            with tc.tile_pool(name="tr_ps", bufs=2, space="PSUM") as tps:
                for m in range(MPC):
                    for f in range(2):
                        ptr = tps.tile([P, P], DT32, space="PSUM", tag="tr")
                        nc.tensor.transpose(
                            ptr[:], h_t[:, D * m + P * f:D * m + P * (f + 1)],
                            ident[:])
                        nc.vector.tensor_copy(
                            hT16[:, NPC * f + P * m:NPC * f + P * (m + 1)],
                            ptr[:])

            ctx_gcn.__exit__(None, None, None)

            # ---- local K^T / V / Q^T, then ONE packed KV AllGather ----
            # kv rows: 0:128 K^T h0 | 128:256 K^T h1 | 256:384 V h0 | 384:512 V h1
            # (V packed as [128, m*128+d] = natural [512, 128] per head)
            qT16 = keep.tile([P, H * NPC], DT16)
            kv_sb = keep.tile([P, 4 * NPC], DT16)
            with tc.tile_pool(name="kv_ps", bufs=3, space="PSUM") as kvps:
                for h in range(H):
                    pq = kvps.tile([P, NPC], DT32, space="PSUM", tag="kv")
                    for k in range(2):
                        nc.tensor.matmul(
                            pq[:],
                            lhsT=winT16[:, 768 * k + P * h:768 * k + P * (h + 1)],
                            rhs=hT16[:, NPC * k:NPC * (k + 1)],
                            start=(k == 0), stop=(k == 1))
                    nc.vector.tensor_scalar(
                        qT16[:, NPC * h:NPC * (h + 1)], pq[:],
                        ipb[:, h:h + 1], None, op0=A.add)
                    pk = kvps.tile([P, NPC], DT32, space="PSUM", tag="kv")
                    for k in range(2):
                        nc.tensor.matmul(
                            pk[:],
                            lhsT=winT16[:, 768 * k + D + P * h:
                                        768 * k + D + P * (h + 1)],
                            rhs=hT16[:, NPC * k:NPC * (k + 1)],
                            start=(k == 0), stop=(k == 1))
                    nc.vector.tensor_scalar(
                        kv_sb[:, NPC * h:NPC * (h + 1)], pk[:],
                        ipb[:, 2 + h:3 + h], None, op0=A.add)
                    for m in range(MPC):
                        pv = kvps.tile([P, P], DT32, space="PSUM", tag="kvv")
                        for k in range(2):
                            nc.tensor.matmul(
                                pv[:],
                                lhsT=hT16[:, NPC * k + P * m:NPC * k + P * (m + 1)],
                                rhs=winT16[:, 768 * k + 2 * D + P * h:
                                            768 * k + 2 * D + P * (h + 1)],
                                start=(k == 0), stop=(k == 1))
                        nc.vector.tensor_copy(
                            kv_sb[:, NPC * (2 + h) + P * m:
                                  NPC * (2 + h) + P * (m + 1)], pv[:])

            kv_bounce = dram.tile([4 * P, NPC], DT16)
            kv_gath = dram.tile([N_CORES * 4 * P, NPC], DT16,
                                addr_space="Shared")
            nc.scalar.dma_start(
                kv_bounce[:].rearrange("(x p) n -> p x n", p=P),
                kv_sb[:].rearrange("p (x n) -> p x n", x=4))
            nc.gpsimd.collective_compute(
                "AllGather", A.bypass,
                replica_groups=[list(range(N_CORES))],
                ins=[kv_bounce.opt()], outs=[kv_gath.opt()])

            # FFN weights stream during the AllGather
            w1T16 = load16(w1T_d, 2 * DFF)
            w2T16 = load16(w2T_d, (DFF // P) * D)

            # ---- load gathered K^T / V ----
            kT16 = keep.tile([P, H * N], DT16)
            v16 = keep.tile([P, H * N], DT16)
            gv = kv_gath[:].rearrange("(g x p) n -> x p g n",
                                      g=N_CORES, x=4, p=P)
            for h in range(H):
                nc.scalar.dma_start(
                    kT16[:, N * h:N * (h + 1)].rearrange(
                        "p (g n) -> p g n", g=N_CORES), gv[h])
                nc.scalar.dma_start(
                    v16[:, N * h:N * (h + 1)].rearrange(
                        "p (g n) -> p g n", g=N_CORES), gv[2 + h])

            # ---- S^T -> exp -> PV + sums ----
            oT16 = keep.tile([P, H * NPC], DT16)
            with tc.tile_pool(name="att_sb", bufs=3) as atsb, \
                 tc.tile_pool(name="att_ps", bufs=1, space="PSUM") as atps, \
                 tc.tile_pool(name="s_ps", bufs=3, space="PSUM") as sps:
                o_ps = [atps.tile([P, NPC], DT32, space="PSUM",
                                  tag=f"o{h}", name=f"o{h}")
                        for h in range(H)]
                sum_ps = [atps.tile([1, NPC], DT32, space="PSUM",
                                    tag=f"sm{h}", name=f"sm{h}")
                          for h in range(H)]
                esum = [None, None]
                for kt in range(KT):
                    for h in range(H):
                        ps_s = sps.tile([P, NPC], DT32, space="PSUM", tag="S")
                        nc.tensor.matmul(
                            ps_s[:],
                            lhsT=kT16[:, N * h + P * kt:N * h + P * (kt + 1)],
                            rhs=qT16[:, NPC * h:NPC * (h + 1)],
                            start=True, stop=True)
                        es = atsb.tile([P, NPC], DT16, tag="es")
                        nc.scalar.activation(es[:], ps_s[:], F.Exp,
                                             scale=INV_SQRT_DH)
                        nc.tensor.matmul(
                            o_ps[h][:],
                            lhsT=v16[:, N * h + P * kt:N * h + P * (kt + 1)],
                            rhs=es[:], start=(kt == 0), stop=(kt == KT - 1))
                        # group exp tiles by 4 on DVE; 1 sums-matmul per group
                        if kt % 4 == 0:
                            eacc = atsb.tile([P, NPC], DT16, tag=f"eac{h}",
                                             name=f"eacc{h}")
                            nc.vector.tensor_copy(eacc[:], es[:])
                            esum[h] = eacc
                        else:
                            nc.vector.tensor_add(esum[h][:], esum[h][:], es[:])
                        if kt % 4 == 3:
                            nc.tensor.matmul(
                                sum_ps[h][:], lhsT=ones16_col[:],
                                rhs=esum[h][:],
                                start=(kt == 3), stop=(kt == KT - 1))

                for h in range(H):
                    rc = atsb.tile([1, NPC], DT32, tag="rc")
                    nc.vector.reciprocal(rc[:], sum_ps[h][:])
                    rc16 = atsb.tile([1, NPC], DT16, tag="rc16")
                    nc.vector.tensor_copy(rc16[:], rc[:])
                    pr = sps.tile([P, NPC], DT32, space="PSUM", tag="S")
                    nc.tensor.matmul(pr[:], lhsT=ones16_row[:], rhs=rc16[:],
                                     start=True, stop=True)
                    rbc = atsb.tile([P, NPC], DT32, tag="rbc")
                    nc.vector.tensor_copy(rbc[:], pr[:])
                    tmp = atsb.tile([P, NPC], DT32, tag="otmp")
                    nc.vector.tensor_mul(tmp[:], o_ps[h][:], rbc[:])
                    nc.vector.tensor_scalar(
                        oT16[:, NPC * h:NPC * (h + 1)], tmp[:],
                        ipb[:, 4 + h:5 + h], None, op0=A.add)

            # ---- o_proj + residual + LN1 (batched) ----
            h1_t = keep.tile([P, MPC * D], DT32)
            h1T16 = keep.tile([P, 2 * NPC], DT16)
            with tc.tile_pool(name="ln_sb", bufs=2) as lsb, \
                 tc.tile_pool(name="op_ps", bufs=2, space="PSUM") as ops:

                def layernorm_all(dst, x_all, g_sl, b_sl, tag):
                    """LN over feature dim for all MPC chunks at once.
                    x_all/dst: [128, MPC*D] fp32 tiles."""
                    mu4 = lsb.tile([P, MPC], DT32, tag=f"{tag}mu")
                    nc.vector.tensor_reduce(
                        mu4[:], x_all[:].rearrange("p (m d) -> p m d", m=MPC),
                        axis=mybir.AxisListType.X, op=A.add)
                    negmu4 = lsb.tile([P, MPC], DT32, tag=f"{tag}nm")
                    nc.vector.tensor_scalar(negmu4[:], mu4[:], -1.0 / D, None,
                                            op0=A.mult)
                    sq4 = lsb.tile([P, D], DT32, tag=f"{tag}sq")
                    ssq4 = lsb.tile([P, MPC], DT32, tag=f"{tag}ss")
                    for m in range(MPC):
                        nc.scalar.activation(sq4[:], x_all[:, D * m:D * (m + 1)],
                                             F.Square, bias=negmu4[:, m:m + 1],
                                             accum_out=ssq4[:, m:m + 1])
                    var4 = lsb.tile([P, MPC], DT32, tag=f"{tag}vr")
                    nc.vector.tensor_scalar(var4[:], ssq4[:], 1.0 / D, EPS,
                                            op0=A.mult, op1=A.add)
                    sd4 = lsb.tile([P, MPC], DT32, tag=f"{tag}sd")
                    nc.scalar.activation(sd4[:], var4[:], F.Sqrt)
                    rstd4 = lsb.tile([P, MPC], DT32, tag=f"{tag}rs")
                    nc.vector.reciprocal(rstd4[:], sd4[:])
                    xc = lsb.tile([P, MPC * D], DT32, tag=f"{tag}xc")
                    for m in range(MPC):
                        nc.vector.tensor_scalar(
                            xc[:, D * m:D * (m + 1)], x_all[:, D * m:D * (m + 1)],
                            negmu4[:, m:m + 1], rstd4[:, m:m + 1],
                            op0=A.add, op1=A.mult)
                    nc.vector.tensor_tensor(
                        xc[:].rearrange("p (m d) -> p m d", m=MPC),
                        xc[:].rearrange("p (m d) -> p m d", m=MPC),
                        bc4(g_sl), op=A.mult)
                    nc.vector.tensor_tensor(
                        dst[:].rearrange("p (m d) -> p m d", m=MPC),
                        xc[:].rearrange("p (m d) -> p m d", m=MPC),
                        bc4(b_sl), op=A.add)

                x1_all = lsb.tile([P, MPC * D], DT32, tag="x1all")
                for m in range(MPC):
                    pa = ops.tile([P, D], DT32, space="PSUM", tag="op")
                    for h in range(H):
                        nc.tensor.matmul(
                            pa[:],
                            lhsT=oT16[:, NPC * h + P * m:NPC * h + P * (m + 1)],
                            rhs=woT16[:, D * h:D * (h + 1)],
                            start=(h == 0), stop=(h == H - 1))
                    nc.vector.scalar_tensor_tensor(
                        x1_all[:, D * m:D * (m + 1)], pa[:], 1.0,
                        h_t[:, D * m:D * (m + 1)], op0=A.mult, op1=A.add)
                nc.vector.tensor_tensor(
                    x1_all[:].rearrange("p (m d) -> p m d", m=MPC),
                    x1_all[:].rearrange("p (m d) -> p m d", m=MPC),
                    bc4(bo_bc), op=A.add)
                layernorm_all(h1_t, x1_all, ln1g_bc, ln1b_bc, "a")

                with tc.tile_pool(name="tr2_ps", bufs=2, space="PSUM") as tps2:
                    for m in range(MPC):
                        for f in range(2):
                            ptr = tps2.tile([P, P], DT32, space="PSUM",
                                            tag="tr2")
                            nc.tensor.transpose(
                                ptr[:],
                                h1_t[:, D * m + P * f:D * m + P * (f + 1)],
                                ident[:])
                            nc.vector.tensor_copy(
                                h1T16[:, NPC * f + P * m:NPC * f + P * (m + 1)],
                                ptr[:])

                # ---- FFN ----
                out_sb = keep.tile([P, MPC * D], DT32)
                ff1T = keep.tile([P, (DFF // P) * NPC], DT16)
                with tc.tile_pool(name="f1_ps", bufs=3, space="PSUM") as fps:
                    for dc in range(DFF // P):
                        pf = fps.tile([P, NPC], DT32, space="PSUM", tag="f1")
                        for k in range(2):
                            nc.tensor.matmul(
                                pf[:],
                                lhsT=w1T16[:, DFF * k + P * dc:
                                           DFF * k + P * (dc + 1)],
                                rhs=h1T16[:, NPC * k:NPC * (k + 1)],
                                start=(k == 0), stop=(k == 1))
                        nc.scalar.activation(
                            ff1T[:, NPC * dc:NPC * (dc + 1)], pf[:], F.Relu,
                            bias=b1t[:, dc:dc + 1])

                x2_all = lsb.tile([P, MPC * D], DT32, tag="x2all")
                with tc.tile_pool(name="f2_ps", bufs=2, space="PSUM") as fps2:
                    for m in range(MPC):
                        pf2 = fps2.tile([P, D], DT32, space="PSUM", tag="f2")
                        for kt2 in range(DFF // P):
                            nc.tensor.matmul(
                                pf2[:],
                                lhsT=ff1T[:, NPC * kt2 + P * m:
                                          NPC * kt2 + P * (m + 1)],
                                rhs=w2T16[:, D * kt2:D * (kt2 + 1)],
                                start=(kt2 == 0), stop=(kt2 == DFF // P - 1))
                        nc.vector.scalar_tensor_tensor(
                            x2_all[:, D * m:D * (m + 1)], pf2[:], 1.0,
                            h1_t[:, D * m:D * (m + 1)], op0=A.mult, op1=A.add)
                nc.vector.tensor_tensor(
                    x2_all[:].rearrange("p (m d) -> p m d", m=MPC),
                    x2_all[:].rearrange("p (m d) -> p m d", m=MPC),
                    bc4(b2_bc), op=A.add)
                layernorm_all(out_sb, x2_all, ln2g_bc, ln2b_bc, "b")
                nc.scalar.dma_start(
                    out_d[:].rearrange("(m p) d -> p m d", p=P),
                    out_sb[:].rearrange("p (m d) -> p m d", m=MPC))

    nc.compile()
    return nc


# ======================= host-side prep =======================

def _prep_inputs(x, edge_index, edge_weight, W_gcn, b_gcn, in_proj_w,
                 in_proj_b, out_proj_w, out_proj_b, lin1_w, lin1_b, lin2_w,
                 lin2_b, ln1_g, ln1_b, ln2_g, ln2_b):
    """Pure index-permutation / layout prep. Returns per-core input maps."""
    x = np.asarray(x, np.float32)
    src = np.asarray(edge_index[0], np.int64)
    dst = np.asarray(edge_index[1], np.int64)
    w = np.asarray(edge_weight, np.float32)

    def wrap128(a):
        # [n*128, m] -> [128, n*m] with col block t <- rows [128t, 128t+128)
        n = a.shape[0] // P
        return np.ascontiguousarray(
            a.reshape(n, P, a.shape[1]).transpose(1, 0, 2).reshape(P, -1))

    iota1024 = np.tile(np.arange(KT * KPAD, dtype=np.float32), (P, 1))
    iota128 = np.tile(np.arange(P, dtype=np.float32), (P, 1))
    ident = np.eye(P, dtype=np.float32)
    bias_stack = np.concatenate([
        np.asarray(v, np.float32).reshape(-1) for v in
        (b_gcn, out_proj_b, lin2_b, ln1_g, ln1_b, ln2_g, ln2_b)
    ]).reshape(1, -1)

    f16 = np.float16
    shared = {
        "wg": wrap128(np.asarray(W_gcn, np.float32)).astype(f16),
        "iota1024": iota1024.astype(f16), "iota128": iota128.astype(f16),
        "ident": ident,
        "winT": wrap128(np.ascontiguousarray(
            np.asarray(in_proj_w, np.float32).T)).astype(f16),
        "ipb": np.ascontiguousarray(
            np.asarray(in_proj_b, np.float32).reshape(6, P).T),
        "woT": wrap128(np.ascontiguousarray(
            np.asarray(out_proj_w, np.float32).T)).astype(f16),
        "w1T": wrap128(np.ascontiguousarray(
            np.asarray(lin1_w, np.float32).T)).astype(f16),
        "b1": np.ascontiguousarray(
            np.asarray(lin1_b, np.float32).reshape(DFF // P, P).T),
        "w2T": wrap128(np.ascontiguousarray(
            np.asarray(lin2_w, np.float32).T)).astype(f16),
        "bias": bias_stack,
    }

    core_of = dst // NPC
    in_maps = []
    for c in range(N_CORES):
        sel = np.nonzero(core_of == c)[0]
        s_c = src[sel]
        d_c = (dst[sel] - NPC * c).astype(np.int64)
        w_c = w[sel]

        w_arr = np.zeros((N, KPAD), np.float32)
        idx_arr = np.full((N, KPAD), -1, np.int16)
        counts = np.zeros(N, np.int32)
        first_slot = {}
        dup_sr, dup_fc, dup_w = [], [], []
        for si, di, wi in zip(s_c.tolist(), d_c.tolist(), w_c.tolist()):
            key = si * NPC + di
            slot = first_slot.get(key)
            if slot is None:
                j = int(counts[si])
                assert j < KPAD, f"KPAD overflow at src {si}"
                counts[si] = j + 1
                w_arr[si, j] = wi
                idx_arr[si, j] = di
                first_slot[key] = j
            else:
                dup_sr.append(si % P)
                dup_fc.append(KPAD * (si // P) + slot)
                dup_w.append(wi)
        assert len(dup_sr) <= NDUP, f"NDUP overflow: {len(dup_sr)}"

        def pad_dup(vals, dtype=np.float32):
            a = np.zeros(NDUP, dtype)
            a[:len(vals)] = vals
            return np.ascontiguousarray(a.reshape(NDUP // P, P).T)

        wbd = np.zeros((NPC, KBD), np.float32)
        cnt2 = np.zeros(NPC, np.int32)
        for di, wi in zip(d_c.tolist(), w_c.tolist()):
            j = int(cnt2[di])
            assert j < KBD, f"KBD overflow at dst {di}"
            wbd[di, j] = wi
            cnt2[di] = j + 1

        in_maps.append({
            **shared,
            "xT": wrap128(np.ascontiguousarray(
                x[NPC * c:NPC * (c + 1)].T)).astype(f16),
            "warr": wrap128(w_arr).astype(f16),
            "idx": wrap128(idx_arr),
            "wbd": wrap128(wbd).astype(f16),
            "dupsr": pad_dup(dup_sr),
            "dupfc": pad_dup(dup_fc),
            "dupw": pad_dup(dup_w),
        })
    return in_maps


# ======================= runner =======================

class _Runner:
    """Persistent-jit SPMD executor (mirrors bass2jax.run_bass_via_pjrt)."""

    def __init__(self, nc):
        import jax
        from jax.sharding import Mesh, PartitionSpec
        from jax.experimental.shard_map import shard_map
        from concourse.bass2jax import (_bass_exec_p, install_neuronx_cc_hook,
                                        partition_id_tensor)
        install_neuronx_cc_hook()
        self.jax = jax
        partition_name = (nc.partition_id_tensor.name
                          if nc.partition_id_tensor else None)
        in_names, out_names, out_avals, zero_outs = [], [], [], []
        for alloc in nc.m.functions[0].allocations:
            if not isinstance(alloc, mybir.MemoryLocationSet):
                continue
            name = alloc.memorylocations[0].name
            if alloc.kind == "ExternalInput":
                if name != partition_name:
                    in_names.append(name)
            elif alloc.kind == "ExternalOutput":
                out_names.append(name)
                shape = tuple(alloc.tensor_shape)
                dtype = mybir.dt.np(alloc.dtype)
                out_avals.append(jax.core.ShapedArray(shape, dtype))
                zero_outs.append(np.zeros(shape, dtype))
        self.in_names, self.out_names = in_names, out_names
        self.out_shapes = [tuple(a.shape) for a in out_avals]
        self.n_params = len(in_names)
        self.zero_outs = zero_outs
        all_in = in_names + out_names
        if partition_name is not None:
            all_in.append(partition_name)

        def _body(*args):
            operands = list(args)
            if partition_name is not None:
                operands.append(partition_id_tensor())
            return tuple(_bass_exec_p.bind(
                *operands, out_avals=tuple(out_avals), in_names=tuple(all_in),
                out_names=tuple(out_names), lowering_input_output_aliases=(),
                sim_require_finite=True, sim_require_nnan=True, nc=nc))

        devices = jax.devices()[:N_CORES]
        self.mesh = Mesh(np.asarray(devices), ("core",))
        nin = self.n_params + len(out_names)
        self.fn = jax.jit(
            shard_map(_body, mesh=self.mesh,
                      in_specs=(PartitionSpec("core"),) * nin,
                      out_specs=(PartitionSpec("core"),) * len(out_names),
                      check_rep=False),
            keep_unused=True)

    def place(self, in_maps):
        import jax
        from jax.sharding import PartitionSpec
        per_core = [[np.asarray(m[n]) for n in self.in_names] for m in in_maps]
        concat = [np.concatenate([per_core[c][i] for c in range(N_CORES)], axis=0)
                  for i in range(self.n_params)]
        zeros = [np.zeros((N_CORES * z.shape[0], *z.shape[1:]), z.dtype)
                 for z in self.zero_outs]
        sh = jax.sharding.NamedSharding(self.mesh, PartitionSpec("core"))
        return [jax.device_put(a, sh) for a in (*concat, *zeros)]

    def run(self, args):
        outs = self.fn(*args)
        self.jax.block_until_ready(outs)
        return outs

    def results(self, outs):
        res = []
        for c in range(N_CORES):
            d = {}
            for i, name in enumerate(self.out_names):
                full = np.asarray(outs[i])
                ps = self.out_shapes[i]
                d[name] = full.reshape((N_CORES,) + ps)[c]
            res.append(d)
        return res


_CACHE = {}


def _get_runner():
    if "runner" not in _CACHE:
        nc = build_kernel()
        _CACHE["nc"] = nc
        _CACHE["runner"] = _Runner(nc)
    return _CACHE["runner"]


def kernel(**inputs) -> np.ndarray:
    runner = _get_runner()
    in_maps = _prep_inputs(**inputs)
    args = runner.place(in_maps)
    outs = runner.run(args)
    res = runner.results(outs)
    return np.concatenate([res[c]["out"] for c in range(N_CORES)], axis=0)
